# revision 21
# baseline (speedup 1.0000x reference)
"""Trainium2 Bass kernel for nn_BackboneDenoisingLayer (graph IPA denoising layer).

Strategy: all dense matmul work (edge MLPs, IPA linear projections, transitions,
out-projections — ~95% of FLOPs) runs on the 8 NeuronCores via Bass/Tile kernels
in a feature-major layout (features on partitions, rows sharded across cores as
matmul free-dim columns).  Irregular glue (edge gathers, segment softmax,
layernorms, quaternion compose) runs on host between launches.
"""

import math
import numpy as np

import concourse.bass as bass
import concourse.tile as tile
import concourse.mybir as mybir
from concourse.bass_utils import run_bass_kernel_spmd
from concourse.vector_clock import ScopedClock

NCORES = 8
P = 128
COLT = 512  # matmul moving free dim / psum bank

C_S, C_LATENT, C_Z, C_HIDDEN = 256, 256, 128, 16
HEADS, QK_PTS, V_PTS = 8, 8, 12
N, E, E_SEQ = 16384, 262144, 131072


# ---------------------------------------------------------------------------
# Tile drain workaround: this walrus build caps sync-wait slots on the SP CTRL
# op; spread the kernel-tail waits across DVE nops instead.
_WAIT_CAP = 1  # this walrus build encodes exactly one sync-wait per ISA inst


def _legalize_waits(nc):
    """Split multi-wait instructions: emit same-engine ENGINE_NOPs carrying the
    overflow waits and splice them immediately before the instruction.  Same
    engine + adjacent position => identical synchronization semantics."""
    snapshots = []
    for f in nc.m.functions:
        for bb in f.blocks:
            snapshots.append((bb, list(bb.instructions)))
    extra = {}  # inst name -> [nop insts]
    for bb, insts in snapshots:
        for inst in insts:
            si = inst.sync_info
            if not si or not si.on_wait or len(si.on_wait) <= _WAIT_CAP:
                continue
            waits = list(si.on_wait)
            keep, overflow = waits[-_WAIT_CAP:], waits[:-_WAIT_CAP]
            nops = []
            eng = nc.engines[inst.engine]
            for w in overflow:
                nop = eng.nop(nofuse=True, hint="wait_split")
                nop.ins.sync_info = mybir.SyncInfo(on_wait=[w], on_update=[])
                nops.append(nop.ins)
            inst.sync_info = mybir.SyncInfo(on_wait=keep,
                                            on_update=list(si.on_update or []))
            extra[inst.name] = nops
    if extra:
        for bb, insts in snapshots:
            new = []
            for inst in insts:
                new.extend(extra.get(inst.name, ()))
                new.append(inst)
            bb.instructions = new


def _patched_drain_and_barrier(self, tick_clock, wait_clock):
    _legalize_waits(self.nc)
    probe = self.nc.vector.engine_nop()
    wait_clock.add_sem_waits(probe.ins, ScopedClock({None: tick_clock.global_clock}))
    waits = list(probe.ins.sync_info.on_wait or []) if probe.ins.sync_info else []
    upds = list(probe.ins.sync_info.on_update or []) if probe.ins.sync_info else []
    probe.ins.sync_info = mybir.SyncInfo(on_wait=waits[:1], on_update=upds)
    for w in waits[1:]:
        nop = self.nc.vector.engine_nop()
        nop.ins.sync_info = mybir.SyncInfo(on_wait=[w], on_update=[])
    self.nc.sync.drain()
    assert self.sems is not None
    self.nc.all_engine_barrier()
    popped = self.nc._tile_sem_poison_stack.pop()
    assert popped is self._sem_poison
    self.nc.clear_and_free_semaphores(list(self.sems.allocated().values()))
    self.nc.all_engine_barrier()


tile.TileContext._drain_and_barrier = _patched_drain_and_barrier

F32 = mybir.dt.float32
F32R = mybir.dt.float32r


def _ktiles(K):
    out = []
    o = 0
    while o < K:
        out.append((o, min(P, K - o)))
        o += P
    return out


def _pack_layout(chains):
    """Column layout of the single packed weight tensor [128, W]."""
    slots = {}
    off = 0
    for ch in chains:
        nm = ch["name"]
        for i, (K, M, _relu) in enumerate(ch["layers"]):
            for kt, (ko, kp) in enumerate(_ktiles(K)):
                slots[(nm, i, kt)] = (off, M, kp)
                off += M
            for mt, (mo, mp) in enumerate(_ktiles(M)):
                slots[(nm, "b", i, mt)] = (off, 1, mp)
                off += 1
    return slots, off


def pack_weights(chains, weights_by_chain):
    slots, wtot = _pack_layout(chains)
    buf = np.zeros((P, wtot), np.float32)
    for ch in chains:
        nm = ch["name"]
        for i, (K, M, _relu) in enumerate(ch["layers"]):
            W, b = weights_by_chain[nm][i]
            for kt, (ko, kp) in enumerate(_ktiles(K)):
                off, width, _ = slots[(nm, i, kt)]
                buf[:kp, off:off + width] = W[ko:ko + kp, :]
            for mt, (mo, mp) in enumerate(_ktiles(M)):
                off, _, _ = slots[(nm, "b", i, mt)]
                buf[:mp, off] = b[mo:mo + mp]
    return buf


def build_program(chains):
    """chains: list of dicts:
      name, K0, ncols (per core), layers=[(K,M,relu)], residual_before_last
    All weights/biases ride in one packed [128, W] tensor ("wpack") so every
    matmul waits on at most 2 semaphores (walrus wait-slot cap workaround).
    Input {name}_in is [K0, ncols] feature-major; output {name}_out [Mlast, ncols].
    """
    nc = bass.Bass("TRN2", target_bir_lowering=False, debug=False)
    slots, wtot = _pack_layout(chains)
    handles = {}
    handles["wpack"] = nc.dram_tensor("wpack", [P, wtot], F32, kind="ExternalInput").ap()
    for ch in chains:
        nm, K0, ncols = ch["name"], ch["K0"], ch["ncols"]
        handles[nm + "_in"] = nc.dram_tensor(nm + "_in", [K0, ncols], F32, kind="ExternalInput").ap()
        Mlast = ch["layers"][-1][1]
        handles[nm + "_out"] = nc.dram_tensor(nm + "_out", [Mlast, ncols], F32, kind="ExternalOutput").ap()

    from contextlib import ExitStack
    with tile.TileContext(nc) as tc, ExitStack() as ctx:
        wpool = ctx.enter_context(tc.tile_pool(name="weights", bufs=1))
        wtile = wpool.tile([P, wtot], F32, tag="wpack")
        nc.sync.dma_start(out=wtile[:], in_=handles["wpack"][:])
        # dummy matmul: absorbs the wpack-DMA wait onto PE's vector clock so
        # real matmuls carry <=1 sync wait (walrus S3_LW wait-slot cap is 1)
        dpsum = ctx.enter_context(tc.tile_pool(name="dpsum", bufs=1, space="PSUM"))
        dummy = dpsum.tile([1, 1], F32, tag="dummy")
        nc.tensor.matmul(out=dummy[:], lhsT=wtile[0:1, 0:1], rhs=wtile[0:1, 0:1],
                         start=True, stop=True)

        def wsb_get(key):
            off, width, kp = slots[key]
            return wtile[:kp, off:off + width]

        wsb = {k: None for k in slots}

        inp = ctx.enter_context(tc.tile_pool(name="inp", bufs=3))
        mid = ctx.enter_context(tc.tile_pool(name="mid", bufs=3))
        psum = ctx.enter_context(tc.tile_pool(name="psum", bufs=4, space="PSUM"))

        for ch in chains:
            nm, K0, ncols = ch["name"], ch["K0"], ch["ncols"]
            layers = ch["layers"]
            resid = ch.get("residual_before_last", False)
            nct = ncols // COLT
            for j in range(nct):
                cs = slice(j * COLT, (j + 1) * COLT)
                in_tiles = []
                for kt, (ko, kp) in enumerate(_ktiles(K0)):
                    t = inp.tile([kp, COLT], F32, tag=f"{nm}in{kt}")
                    nc.sync.dma_start(out=t[:], in_=handles[nm + "_in"][ko:ko + kp, cs])
                    in_tiles.append(t)
                cur = in_tiles
                for li, (K, M, relu) in enumerate(layers):
                    if resid and li == len(layers) - 1:
                        for kt in range(len(cur)):
                            nc.vector.tensor_add(out=cur[kt][:], in0=cur[kt][:], in1=in_tiles[kt][:])
                    outs = []
                    kts = _ktiles(K)
                    assert len(kts) == len(cur), (nm, li, K, len(cur))
                    for mt, (mo, mp) in enumerate(_ktiles(M)):
                        ps = psum.tile([mp, COLT], F32, tag="ps")
                        for kt, (ko, kp) in enumerate(kts):
                            nc.tensor.matmul(
                                out=ps[:],
                                lhsT=wsb_get((nm, li, kt))[:, mo:mo + mp],
                                rhs=cur[kt][:],
                                start=(kt == 0),
                                stop=(kt == len(kts) - 1),
                            )
                        ot = mid.tile([mp, COLT], F32, tag=f"{nm}l{li}m{mt}")
                        nc.scalar.activation(
                            out=ot[:],
                            in_=ps[:],
                            func=(mybir.ActivationFunctionType.Relu if relu
                                  else mybir.ActivationFunctionType.Identity),
                            bias=wsb_get((nm, "b", li, mt)),
                        )
                        outs.append(ot)
                    cur = outs
                Mlast = layers[-1][1]
                for mt, (mo, mp) in enumerate(_ktiles(Mlast)):
                    nc.sync.dma_start(out=handles[nm + "_out"][mo:mo + mp, cs], in_=cur[mt][:])
    return nc


_PROG_CACHE = {}
DEVICE_S = []  # wall-clock seconds of each device launch (incl. transfers)


def run_chains(key, chains, inputs_by_chain, weights_by_chain):
    """inputs_by_chain: {name: full [K0, NC_total] fp32 array} sharded by cols.
    weights_by_chain: {name: [(W,b),...]}.  Returns {name: [Mlast, NC_total]}."""
    if key not in _PROG_CACHE:
        _PROG_CACHE[key] = build_program(chains)
    nc = _PROG_CACHE[key]
    wpack = pack_weights(chains, weights_by_chain)
    in_maps = []
    for c in range(NCORES):
        m = {"wpack": wpack}
        for ch in chains:
            nm, ncols = ch["name"], ch["ncols"]
            full = inputs_by_chain[nm]
            m[nm + "_in"] = np.ascontiguousarray(full[:, c * ncols:(c + 1) * ncols])
        in_maps.append(m)
    import time as _time
    _t0 = _time.time()
    res = run_bass_kernel_spmd(nc, in_maps, list(range(NCORES)))
    DEVICE_S.append(_time.time() - _t0)
    out = {}
    for ch in chains:
        nm = ch["name"]
        out[nm] = np.concatenate([res.results[c][nm + "_out"] for c in range(NCORES)], axis=1)
    return out


# ---------------------------------------------------------------------------
# host helpers

def _ln(x, g, b, eps=1e-5):
    m = x.mean(-1, keepdims=True)
    v = x.var(-1, keepdims=True)
    return (x - m) / np.sqrt(v + eps) * g + b


def _T(x):
    return np.ascontiguousarray(np.asarray(x, np.float32).T)


def _wb(p):
    return (np.asarray(p["w"], np.float32), np.asarray(p["b"], np.float32))


class _SegHelper:
    def __init__(self, row, n):
        self.order = np.argsort(row, kind="stable")
        self.row_s = row[self.order]
        counts = np.bincount(row, minlength=n)
        starts = np.concatenate([[0], np.cumsum(counts)[:-1]])
        self.nz = counts > 0
        self.starts_nz = starts[self.nz].astype(np.int64)
        self.n = n

    def sum(self, x_sorted):
        out = np.zeros((self.n,) + x_sorted.shape[1:], np.float32)
        if len(self.starts_nz):
            out[self.nz] = np.add.reduceat(x_sorted, self.starts_nz, axis=0)
        return out

    def max(self, x_sorted):
        out = np.zeros((self.n,) + x_sorted.shape[1:], np.float32)
        if len(self.starts_nz):
            red = np.maximum.reduceat(x_sorted, self.starts_nz, axis=0)
            out[self.nz] = red
        return out


def _ipa_edge_stage(q, k, v, qp_g, kp_g, vp_g, z_s, b_s, row_s, col_s, seg, rot, trans, mask, gamma):
    """All edge arrays (_s) already in row-sorted order. Returns feats [N,1536]."""
    w_c = math.sqrt(2.0 / (9.0 * QK_PTS))
    w_l = math.sqrt(1.0 / 3.0)
    qk = np.einsum("ehc,ehc->eh", q[row_s], k[col_s], optimize=True) / math.sqrt(C_HIDDEN)
    qpr = qp_g.reshape(N, HEADS, QK_PTS * 3)
    kpr = kp_g.reshape(N, HEADS, QK_PTS * 3)
    nq = (qpr ** 2).sum(-1)
    nk = (kpr ** 2).sum(-1)
    cross = np.einsum("ehc,ehc->eh", qpr[row_s], kpr[col_s], optimize=True)
    d2 = nq[row_s] + nk[col_s] - 2.0 * cross
    logits = w_l * (qk + b_s) - 0.5 * w_l * w_c * gamma[None, :] * d2
    logits = logits + (mask[col_s] - 1.0)[:, None] * 1e5
    mx = seg.max(logits)  # empty segments -> 0, matching ref's isfinite fixup
    ex = np.exp(logits - mx[row_s])
    den = seg.sum(ex)
    a = ex / (den[row_s] + 1e-9)
    o = seg.sum(a[..., None] * v[col_s]).reshape(N, -1)
    o_pt_g = seg.sum(a[:, :, None, None] * vp_g[col_s])  # [N,H,VP,3]
    o_pt = np.einsum("nji,nhpj->nhpi", rot, o_pt_g - trans[:, None, None, :], optimize=True)
    o_pt_norm = np.sqrt((o_pt ** 2).sum(-1) + 1e-8)
    o_pair = np.empty((N, HEADS, C_Z), np.float32)
    for h in range(HEADS):
        o_pair[:, h] = seg.sum(a[:, h, None] * z_s)
    feats = np.concatenate(
        [o, o_pt.reshape(N, -1), o_pt_norm.reshape(N, -1), o_pair.reshape(N, -1)], axis=-1
    ).astype(np.float32)
    return feats


def _to_global(pts, rot, trans):
    # pts [N,H,P,3] local -> global
    return np.einsum("nij,nhpj->nhpi", rot, pts, optimize=True) + trans[:, None, None, :]


def _quat_to_rot(q):
    w, x, y, z = q[..., 0], q[..., 1], q[..., 2], q[..., 3]
    r0 = np.stack([1 - 2 * (y * y + z * z), 2 * (x * y - w * z), 2 * (x * z + w * y)], -1)
    r1 = np.stack([2 * (x * y + w * z), 1 - 2 * (x * x + z * z), 2 * (y * z - w * x)], -1)
    r2 = np.stack([2 * (x * z - w * y), 2 * (y * z + w * x), 1 - 2 * (x * x + y * y)], -1)
    return np.stack([r0, r1, r2], -2)


def _ipa_linears(key, node, z_s, ipa_p, ncols_node):
    """Run IPA projection linears on device. Returns q,k,v,[local pts]."""
    Ws, bs = [], []
    for name in ("q", "k", "v", "q_pts", "k_pts", "v_pts"):
        W, b = _wb(ipa_p[name])
        Ws.append(W)
        bs.append(b)
    Wcat = np.concatenate(Ws, axis=1)
    bcat = np.concatenate(bs)
    chains = [dict(name="ipal", K0=C_S, ncols=ncols_node, layers=[(C_S, Wcat.shape[1], False)])]
    out = run_chains(("ipal", Wcat.shape[1]), chains, {"ipal": _T(node)}, {"ipal": [(Wcat, bcat)]})
    full = out["ipal"].T  # [N, 1056]
    hc = HEADS * C_HIDDEN
    ofs = np.cumsum([0, hc, hc, hc, HEADS * QK_PTS * 3, HEADS * QK_PTS * 3, HEADS * V_PTS * 3])
    q = full[:, ofs[0]:ofs[1]].reshape(N, HEADS, C_HIDDEN)
    k = full[:, ofs[1]:ofs[2]].reshape(N, HEADS, C_HIDDEN)
    v = full[:, ofs[2]:ofs[3]].reshape(N, HEADS, C_HIDDEN)
    qp = full[:, ofs[3]:ofs[4]].reshape(N, HEADS, QK_PTS, 3)
    kp = full[:, ofs[4]:ofs[5]].reshape(N, HEADS, QK_PTS, 3)
    vp = full[:, ofs[5]:ofs[6]].reshape(N, HEADS, V_PTS, 3)
    return q, k, v, qp, kp, vp


def kernel(node_features, rot, trans, latent_features, edge_features,
           new_seq_edge_inputs, seq_edge_features, edge_index, seq_edge_index,
           res_mask, noising_mask, params):
    p = params
    nf = np.asarray(node_features, np.float32)
    rot = np.asarray(rot, np.float32)
    trans = np.asarray(trans, np.float32)
    latent = np.asarray(latent_features, np.float32)
    ef = np.asarray(edge_features, np.float32)
    nsei = np.asarray(new_seq_edge_inputs, np.float32)
    sef = np.asarray(seq_edge_features, np.float32)
    eidx = np.asarray(edge_index, np.int32)
    seidx = np.asarray(seq_edge_index, np.int32)
    mask = np.asarray(res_mask, np.float32)
    noise = np.asarray(noising_mask, np.float32)

    # sort both edge lists by destination (row) once; un-permute outputs at the end
    seg_sp = _SegHelper(eidx[0], N)
    seg_sq = _SegHelper(seidx[0], N)
    o_sp, o_sq = seg_sp.order, seg_sq.order
    r0s, c0s = eidx[0][o_sp], eidx[1][o_sp]
    s0s, s1s = seidx[0][o_sq], seidx[1][o_sq]
    ef_s = ef[o_sp]
    nsei_s = nsei[o_sq]
    sef_s = sef[o_sq]

    ncols_e, ncols_sq, ncols_n = E // NCORES, E_SEQ // NCORES, N // NCORES

    # ---- phase 1: edge embed MLP, seq edge update MLP, latent->node MLP
    ee = p["edge_embed"]
    su = p["seq_edge_update"]
    l2n = p["lat2node"]
    edge_in = np.concatenate([ef_s, nf[r0s], nf[c0s]], axis=1)
    seq_in = np.concatenate([nsei_s, nf[s0s], nf[s1s]], axis=1)
    lat_in = np.concatenate([latent, nf], axis=1)
    ein = edge_in.shape[1]
    ch1 = [
        dict(name="ee", K0=ein, ncols=ncols_e,
             layers=[(ein, C_Z, True), (C_Z, C_Z, True), (C_Z, C_Z, False)]),
        dict(name="su", K0=ein, ncols=ncols_sq,
             layers=[(ein, C_Z, True), (C_Z, C_Z, True), (C_Z, C_Z, False)]),
        dict(name="l2n", K0=2 * C_S, ncols=ncols_n,
             layers=[(2 * C_S, C_S, True), (C_S, C_S, True), (C_S, C_S, False)]),
    ]
    w1 = {
        "ee": [_wb(ee["l1"]), _wb(ee["l2"]), _wb(ee["l3"])],
        "su": [_wb(su["l1"]), _wb(su["l2"]), _wb(su["l3"])],
        "l2n": [_wb(l2n["l1"]), _wb(l2n["l2"]), _wb(l2n["l3"])],
    }
    out1 = run_chains("p1", ch1, {"ee": _T(edge_in), "su": _T(seq_in), "l2n": _T(lat_in)}, w1)

    edge_out_s = _ln(out1["ee"].T, np.asarray(ee["ln"]["g"], np.float32),
                     np.asarray(ee["ln"]["b"], np.float32))
    seq_edge_s = _ln(sef_s + out1["su"].T, np.asarray(p["seq_edge_ln"]["g"], np.float32),
                     np.asarray(p["seq_edge_ln"]["b"], np.float32))
    node = _ln(nf + out1["l2n"].T, np.asarray(p["ln_s0"]["g"], np.float32),
               np.asarray(p["ln_s0"]["b"], np.float32))

    # ---- IPA (spatial), then IPA (sequence)
    for which, z_s, row_s, col_s, seg in (
        ("ipa_sp", edge_out_s, r0s, c0s, seg_sp),
        ("ipa_seq", seq_edge_s, s0s, s1s, seg_sq),
    ):
        ip = p[which]
        q, k, v, qp, kp, vp = _ipa_linears(which, node, z_s, ip, ncols_n)
        qp_g = _to_global(qp, rot, trans)
        kp_g = _to_global(kp, rot, trans)
        vp_g = _to_global(vp, rot, trans)
        Wb, bb_ = _wb(ip["bias"])
        b_s = z_s @ Wb + bb_
        gamma = np.log1p(np.exp(np.asarray(ip["head_w"], np.float32)))
        feats = _ipa_edge_stage(q, k, v, qp_g, kp_g, vp_g, z_s, b_s, row_s, col_s,
                                seg, rot, trans, mask, gamma)
        Wo, bo = _wb(ip["out"])
        cho = [dict(name="out", K0=feats.shape[1], ncols=ncols_n,
                    layers=[(feats.shape[1], C_S, False)])]
        upd = run_chains(("ipaout", feats.shape[1]), cho, {"out": _T(feats)},
                         {"out": [(Wo, bo)]})["out"].T
        lnp = p["ln_s1"] if which == "ipa_sp" else p["ln_s2"]
        node = _ln(node + upd * mask[:, None], np.asarray(lnp["g"], np.float32),
                   np.asarray(lnp["b"], np.float32))

    # ---- node transition
    nt = p["node_trans"]
    ch6 = [dict(name="nt", K0=C_S, ncols=ncols_n,
                layers=[(C_S, C_S, True), (C_S, C_S, True), (C_S, C_S, False)])]
    out6 = run_chains("p6", ch6, {"nt": _T(node)},
                      {"nt": [_wb(nt["l1"]), _wb(nt["l2"]), _wb(nt["l3"])]})
    node = _ln(node + out6["nt"].T, np.asarray(nt["ln"]["g"], np.float32),
               np.asarray(nt["ln"]["b"], np.float32))
    node = node * mask[:, None]

    # ---- backbone update (host: tiny) + frame compose
    Wbb, bbb = _wb(p["bb"])
    upd6 = ((node * noise[:, None]) @ Wbb + bbb) * noise[:, None]
    qv = np.concatenate([np.ones_like(upd6[:, :1]), upd6[:, :3]], axis=-1)
    qv = qv / np.linalg.norm(qv, axis=-1, keepdims=True)
    new_rot = np.einsum("nij,njk->nik", rot, _quat_to_rot(qv), optimize=True)
    new_trans = trans + np.einsum("nij,nj->ni", rot, upd6[:, 3:], optimize=True)

    # ---- edge transition (on seq edges) + node->latent
    et = p["edge_trans"]
    n2l = p["node2lat"]
    hid = C_Z + 2 * (C_S // 2)
    n2l_in = np.concatenate([latent, node], axis=1)
    ch7 = [
        dict(name="h", K0=C_S, ncols=ncols_n, layers=[(C_S, C_S // 2, False)]),
        dict(name="n2l", K0=C_S + C_LATENT, ncols=ncols_n,
             layers=[(C_S + C_LATENT, C_LATENT, True), (C_LATENT, C_LATENT, True),
                     (C_LATENT, C_LATENT, False)]),
    ]
    out7 = run_chains("p7", ch7, {"h": _T(node), "n2l": _T(n2l_in)},
                      {"h": [_wb(et["init"])],
                       "n2l": [_wb(n2l["l1"]), _wb(n2l["l2"]), _wb(n2l["l3"])]})
    h = out7["h"].T
    latent_out = latent + out7["n2l"].T

    x_et = np.concatenate([seq_edge_s, h[s0s], h[s1s]], axis=1)
    ch8 = [dict(name="et", K0=hid, ncols=ncols_sq,
                layers=[(hid, hid, True), (hid, hid, True), (hid, C_Z, False)],
                residual_before_last=True)]
    out8 = run_chains("p8", ch8, {"et": _T(x_et)},
                      {"et": [_wb(et["t1"]), _wb(et["t2"]), _wb(et["fin"])]})
    seq_final_s = _ln(out8["et"].T, np.asarray(et["ln"]["g"], np.float32),
                      np.asarray(et["ln"]["b"], np.float32))

    # un-permute edge outputs back to original edge order
    edge_out = np.empty_like(edge_out_s)
    edge_out[o_sp] = edge_out_s
    seq_final = np.empty_like(seq_final_s)
    seq_final[o_sq] = seq_final_s

    return (node.astype(np.float32), new_rot.astype(np.float32),
            new_trans.astype(np.float32), edge_out.astype(np.float32),
            seq_final.astype(np.float32), latent_out.astype(np.float32))


# revision 22
# speedup vs baseline: 2.0565x; 2.0565x over previous
"""Trainium2 Bass kernel for nn_BackboneDenoisingLayer (graph IPA denoising layer).

Strategy: all dense matmul work (edge MLPs, IPA linear projections, transitions,
out-projections — ~95% of FLOPs) runs on the 8 NeuronCores via Bass/Tile kernels
in a feature-major layout (features on partitions, rows sharded across cores as
matmul free-dim columns).  Irregular glue (edge gathers, segment softmax,
layernorms, quaternion compose) runs on host between launches.
"""

import math
import numpy as np

import concourse.bass as bass
import concourse.tile as tile
import concourse.mybir as mybir
from concourse.bass_utils import run_bass_kernel_spmd
from concourse.vector_clock import ScopedClock

NCORES = 8
P = 128
COLT = 512  # matmul moving free dim / psum bank

C_S, C_LATENT, C_Z, C_HIDDEN = 256, 256, 128, 16
HEADS, QK_PTS, V_PTS = 8, 8, 12
N, E, E_SEQ = 16384, 262144, 131072


# ---------------------------------------------------------------------------
# Tile drain workaround: this walrus build caps sync-wait slots on the SP CTRL
# op; spread the kernel-tail waits across DVE nops instead.
_WAIT_CAP = 1  # this walrus build encodes exactly one sync-wait per ISA inst


def _legalize_waits(nc):
    """Split multi-wait instructions: emit same-engine ENGINE_NOPs carrying the
    overflow waits and splice them immediately before the instruction.  Same
    engine + adjacent position => identical synchronization semantics."""
    snapshots = []
    for f in nc.m.functions:
        for bb in f.blocks:
            snapshots.append((bb, list(bb.instructions)))
    extra = {}  # inst name -> [nop insts]
    for bb, insts in snapshots:
        for inst in insts:
            si = inst.sync_info
            if not si or not si.on_wait or len(si.on_wait) <= _WAIT_CAP:
                continue
            waits = list(si.on_wait)
            keep, overflow = waits[-_WAIT_CAP:], waits[:-_WAIT_CAP]
            nops = []
            eng = nc.engines[inst.engine]
            for w in overflow:
                nop = eng.nop(nofuse=True, hint="wait_split")
                nop.ins.sync_info = mybir.SyncInfo(on_wait=[w], on_update=[])
                nops.append(nop.ins)
            inst.sync_info = mybir.SyncInfo(on_wait=keep,
                                            on_update=list(si.on_update or []))
            extra[inst.name] = nops
    if extra:
        for bb, insts in snapshots:
            new = []
            for inst in insts:
                new.extend(extra.get(inst.name, ()))
                new.append(inst)
            bb.instructions = new


def _patched_drain_and_barrier(self, tick_clock, wait_clock):
    _legalize_waits(self.nc)
    probe = self.nc.vector.engine_nop()
    wait_clock.add_sem_waits(probe.ins, ScopedClock({None: tick_clock.global_clock}))
    waits = list(probe.ins.sync_info.on_wait or []) if probe.ins.sync_info else []
    upds = list(probe.ins.sync_info.on_update or []) if probe.ins.sync_info else []
    probe.ins.sync_info = mybir.SyncInfo(on_wait=waits[:1], on_update=upds)
    for w in waits[1:]:
        nop = self.nc.vector.engine_nop()
        nop.ins.sync_info = mybir.SyncInfo(on_wait=[w], on_update=[])
    self.nc.sync.drain()
    assert self.sems is not None
    self.nc.all_engine_barrier()
    popped = self.nc._tile_sem_poison_stack.pop()
    assert popped is self._sem_poison
    self.nc.clear_and_free_semaphores(list(self.sems.allocated().values()))
    self.nc.all_engine_barrier()


tile.TileContext._drain_and_barrier = _patched_drain_and_barrier

F32 = mybir.dt.float32
BF16 = mybir.dt.bfloat16
import ml_dtypes
NPBF16 = ml_dtypes.bfloat16


def _ktiles(K):
    out = []
    o = 0
    while o < K:
        out.append((o, min(P, K - o)))
        o += P
    return out


def _pack_layout(chains):
    """Column layout of the single packed weight tensor [128, W]."""
    slots = {}
    off = 0
    for ch in chains:
        nm = ch["name"]
        for i, (K, M, _relu) in enumerate(ch["layers"]):
            for kt, (ko, kp) in enumerate(_ktiles(K)):
                slots[(nm, i, kt)] = (off, M, kp)
                off += M
            for mt, (mo, mp) in enumerate(_ktiles(M)):
                slots[(nm, "b", i, mt)] = (off, 1, mp)
                off += 1
    return slots, off


def pack_weights(chains, weights_by_chain):
    slots, wtot = _pack_layout(chains)
    buf = np.zeros((P, wtot), NPBF16)
    for ch in chains:
        nm = ch["name"]
        for i, (K, M, _relu) in enumerate(ch["layers"]):
            W, b = weights_by_chain[nm][i]
            for kt, (ko, kp) in enumerate(_ktiles(K)):
                off, width, _ = slots[(nm, i, kt)]
                buf[:kp, off:off + width] = W[ko:ko + kp, :]
            for mt, (mo, mp) in enumerate(_ktiles(M)):
                off, _, _ = slots[(nm, "b", i, mt)]
                buf[:mp, off] = b[mo:mo + mp]
    return buf


def build_program(chains):
    """chains: list of dicts:
      name, K0, ncols (per core), layers=[(K,M,relu)], residual_before_last
    All weights/biases ride in one packed [128, W] tensor ("wpack") so every
    matmul waits on at most 2 semaphores (walrus wait-slot cap workaround).
    Input {name}_in is [K0, ncols] feature-major; output {name}_out [Mlast, ncols].
    """
    nc = bass.Bass("TRN2", target_bir_lowering=False, debug=False)
    slots, wtot = _pack_layout(chains)
    handles = {}
    handles["wpack"] = nc.dram_tensor("wpack", [P, wtot], BF16, kind="ExternalInput").ap()
    for ch in chains:
        nm, K0, ncols = ch["name"], ch["K0"], ch["ncols"]
        handles[nm + "_in"] = nc.dram_tensor(nm + "_in", [K0, ncols], BF16, kind="ExternalInput").ap()
        Mlast = ch["layers"][-1][1]
        handles[nm + "_out"] = nc.dram_tensor(nm + "_out", [Mlast, ncols], BF16, kind="ExternalOutput").ap()

    from contextlib import ExitStack
    with tile.TileContext(nc) as tc, ExitStack() as ctx:
        wpool = ctx.enter_context(tc.tile_pool(name="weights", bufs=1))
        wtile = wpool.tile([P, wtot], BF16, tag="wpack")
        nc.sync.dma_start(out=wtile[:], in_=handles["wpack"][:])
        # dummy matmul: absorbs the wpack-DMA wait onto PE's vector clock so
        # real matmuls carry <=1 sync wait (walrus S3_LW wait-slot cap is 1)
        dpsum = ctx.enter_context(tc.tile_pool(name="dpsum", bufs=1, space="PSUM"))
        dummy = dpsum.tile([1, 1], F32, tag="dummy")
        nc.tensor.matmul(out=dummy[:], lhsT=wtile[0:1, 0:1], rhs=wtile[0:1, 0:1],
                         start=True, stop=True)

        def wsb_get(key):
            off, width, kp = slots[key]
            return wtile[:kp, off:off + width]

        wsb = {k: None for k in slots}

        inp = ctx.enter_context(tc.tile_pool(name="inp", bufs=3))
        mid = ctx.enter_context(tc.tile_pool(name="mid", bufs=3))
        psum = ctx.enter_context(tc.tile_pool(name="psum", bufs=4, space="PSUM"))

        for ch in chains:
            nm, K0, ncols = ch["name"], ch["K0"], ch["ncols"]
            layers = ch["layers"]
            resid = ch.get("residual_before_last", False)
            nct = ncols // COLT
            for j in range(nct):
                cs = slice(j * COLT, (j + 1) * COLT)
                in_tiles = []
                for kt, (ko, kp) in enumerate(_ktiles(K0)):
                    t = inp.tile([kp, COLT], BF16, tag=f"{nm}in{kt}")
                    nc.sync.dma_start(out=t[:], in_=handles[nm + "_in"][ko:ko + kp, cs])
                    in_tiles.append(t)
                cur = in_tiles
                for li, (K, M, relu) in enumerate(layers):
                    if resid and li == len(layers) - 1:
                        for kt in range(len(cur)):
                            nc.vector.tensor_add(out=cur[kt][:], in0=cur[kt][:], in1=in_tiles[kt][:])
                    outs = []
                    kts = _ktiles(K)
                    assert len(kts) == len(cur), (nm, li, K, len(cur))
                    for mt, (mo, mp) in enumerate(_ktiles(M)):
                        ps = psum.tile([mp, COLT], F32, tag="ps")
                        for kt, (ko, kp) in enumerate(kts):
                            nc.tensor.matmul(
                                out=ps[:],
                                lhsT=wsb_get((nm, li, kt))[:, mo:mo + mp],
                                rhs=cur[kt][:],
                                start=(kt == 0),
                                stop=(kt == len(kts) - 1),
                            )
                        ot = mid.tile([mp, COLT], BF16, tag=f"{nm}l{li}m{mt}")
                        nc.scalar.activation(
                            out=ot[:],
                            in_=ps[:],
                            func=(mybir.ActivationFunctionType.Relu if relu
                                  else mybir.ActivationFunctionType.Identity),
                            bias=wsb_get((nm, "b", li, mt)),
                        )
                        outs.append(ot)
                    cur = outs
                Mlast = layers[-1][1]
                for mt, (mo, mp) in enumerate(_ktiles(Mlast)):
                    nc.sync.dma_start(out=handles[nm + "_out"][mo:mo + mp, cs], in_=cur[mt][:])
    return nc


_PROG_CACHE = {}
DEVICE_S = []  # wall-clock seconds of each device launch (incl. transfers)


def run_chains(key, chains, inputs_by_chain, weights_by_chain):
    """inputs_by_chain: {name: full [K0, NC_total] fp32 array} sharded by cols.
    weights_by_chain: {name: [(W,b),...]}.  Returns {name: [Mlast, NC_total]}."""
    if key not in _PROG_CACHE:
        _PROG_CACHE[key] = build_program(chains)
    nc = _PROG_CACHE[key]
    wpack = pack_weights(chains, weights_by_chain)
    in_maps = []
    for c in range(NCORES):
        m = {"wpack": wpack}
        for ch in chains:
            nm, ncols = ch["name"], ch["ncols"]
            full = inputs_by_chain[nm]
            m[nm + "_in"] = np.ascontiguousarray(full[:, c * ncols:(c + 1) * ncols]).astype(NPBF16)
        in_maps.append(m)
    import time as _time
    _t0 = _time.time()
    res = run_bass_kernel_spmd(nc, in_maps, list(range(NCORES)))
    DEVICE_S.append(_time.time() - _t0)
    out = {}
    for ch in chains:
        nm = ch["name"]
        out[nm] = np.concatenate([res.results[c][nm + "_out"].astype(np.float32) for c in range(NCORES)], axis=1)
    return out


# ---------------------------------------------------------------------------
# host helpers

def _ln(x, g, b, eps=1e-5):
    m = x.mean(-1, keepdims=True)
    v = x.var(-1, keepdims=True)
    return (x - m) / np.sqrt(v + eps) * g + b


def _T(x):
    return np.ascontiguousarray(np.asarray(x, np.float32).T)


def _wb(p):
    return (np.asarray(p["w"], np.float32), np.asarray(p["b"], np.float32))


class _SegHelper:
    def __init__(self, row, n):
        self.order = np.argsort(row, kind="stable")
        self.row_s = row[self.order]
        counts = np.bincount(row, minlength=n)
        starts = np.concatenate([[0], np.cumsum(counts)[:-1]])
        self.nz = counts > 0
        self.starts_nz = starts[self.nz].astype(np.int64)
        self.n = n

    def sum(self, x_sorted):
        out = np.zeros((self.n,) + x_sorted.shape[1:], np.float32)
        if len(self.starts_nz):
            out[self.nz] = np.add.reduceat(x_sorted, self.starts_nz, axis=0)
        return out

    def max(self, x_sorted):
        out = np.zeros((self.n,) + x_sorted.shape[1:], np.float32)
        if len(self.starts_nz):
            red = np.maximum.reduceat(x_sorted, self.starts_nz, axis=0)
            out[self.nz] = red
        return out


def _ipa_edge_stage(q, k, v, qp_g, kp_g, vp_g, z_s, b_s, row_s, col_s, seg, rot, trans, mask, gamma):
    """All edge arrays (_s) already in row-sorted order. Returns feats [N,1536]."""
    w_c = math.sqrt(2.0 / (9.0 * QK_PTS))
    w_l = math.sqrt(1.0 / 3.0)
    qk = np.einsum("ehc,ehc->eh", q[row_s], k[col_s], optimize=True) / math.sqrt(C_HIDDEN)
    qpr = qp_g.reshape(N, HEADS, QK_PTS * 3)
    kpr = kp_g.reshape(N, HEADS, QK_PTS * 3)
    nq = (qpr ** 2).sum(-1)
    nk = (kpr ** 2).sum(-1)
    cross = np.einsum("ehc,ehc->eh", qpr[row_s], kpr[col_s], optimize=True)
    d2 = nq[row_s] + nk[col_s] - 2.0 * cross
    logits = w_l * (qk + b_s) - 0.5 * w_l * w_c * gamma[None, :] * d2
    logits = logits + (mask[col_s] - 1.0)[:, None] * 1e5
    mx = seg.max(logits)  # empty segments -> 0, matching ref's isfinite fixup
    ex = np.exp(logits - mx[row_s])
    den = seg.sum(ex)
    a = ex / (den[row_s] + 1e-9)
    o = seg.sum(a[..., None] * v[col_s]).reshape(N, -1)
    o_pt_g = seg.sum(a[:, :, None, None] * vp_g[col_s])  # [N,H,VP,3]
    o_pt = np.einsum("nji,nhpj->nhpi", rot, o_pt_g - trans[:, None, None, :], optimize=True)
    o_pt_norm = np.sqrt((o_pt ** 2).sum(-1) + 1e-8)
    o_pair = np.empty((N, HEADS, C_Z), np.float32)
    for h in range(HEADS):
        o_pair[:, h] = seg.sum(a[:, h, None] * z_s)
    feats = np.concatenate(
        [o, o_pt.reshape(N, -1), o_pt_norm.reshape(N, -1), o_pair.reshape(N, -1)], axis=-1
    ).astype(np.float32)
    return feats


def _to_global(pts, rot, trans):
    # pts [N,H,P,3] local -> global
    return np.einsum("nij,nhpj->nhpi", rot, pts, optimize=True) + trans[:, None, None, :]


def _quat_to_rot(q):
    w, x, y, z = q[..., 0], q[..., 1], q[..., 2], q[..., 3]
    r0 = np.stack([1 - 2 * (y * y + z * z), 2 * (x * y - w * z), 2 * (x * z + w * y)], -1)
    r1 = np.stack([2 * (x * y + w * z), 1 - 2 * (x * x + z * z), 2 * (y * z - w * x)], -1)
    r2 = np.stack([2 * (x * z - w * y), 2 * (y * z + w * x), 1 - 2 * (x * x + y * y)], -1)
    return np.stack([r0, r1, r2], -2)


def _ipa_linears(key, node, z_s, ipa_p, ncols_node):
    """Run IPA projection linears on device. Returns q,k,v,[local pts]."""
    Ws, bs = [], []
    for name in ("q", "k", "v", "q_pts", "k_pts", "v_pts"):
        W, b = _wb(ipa_p[name])
        Ws.append(W)
        bs.append(b)
    Wcat = np.concatenate(Ws, axis=1)
    bcat = np.concatenate(bs)
    chains = [dict(name="ipal", K0=C_S, ncols=ncols_node, layers=[(C_S, Wcat.shape[1], False)])]
    out = run_chains(("ipal", Wcat.shape[1]), chains, {"ipal": _T(node)}, {"ipal": [(Wcat, bcat)]})
    full = out["ipal"].T  # [N, 1056]
    hc = HEADS * C_HIDDEN
    ofs = np.cumsum([0, hc, hc, hc, HEADS * QK_PTS * 3, HEADS * QK_PTS * 3, HEADS * V_PTS * 3])
    q = full[:, ofs[0]:ofs[1]].reshape(N, HEADS, C_HIDDEN)
    k = full[:, ofs[1]:ofs[2]].reshape(N, HEADS, C_HIDDEN)
    v = full[:, ofs[2]:ofs[3]].reshape(N, HEADS, C_HIDDEN)
    qp = full[:, ofs[3]:ofs[4]].reshape(N, HEADS, QK_PTS, 3)
    kp = full[:, ofs[4]:ofs[5]].reshape(N, HEADS, QK_PTS, 3)
    vp = full[:, ofs[5]:ofs[6]].reshape(N, HEADS, V_PTS, 3)
    return q, k, v, qp, kp, vp


def kernel(node_features, rot, trans, latent_features, edge_features,
           new_seq_edge_inputs, seq_edge_features, edge_index, seq_edge_index,
           res_mask, noising_mask, params):
    p = params
    nf = np.asarray(node_features, np.float32)
    rot = np.asarray(rot, np.float32)
    trans = np.asarray(trans, np.float32)
    latent = np.asarray(latent_features, np.float32)
    ef = np.asarray(edge_features, np.float32)
    nsei = np.asarray(new_seq_edge_inputs, np.float32)
    sef = np.asarray(seq_edge_features, np.float32)
    eidx = np.asarray(edge_index, np.int32)
    seidx = np.asarray(seq_edge_index, np.int32)
    mask = np.asarray(res_mask, np.float32)
    noise = np.asarray(noising_mask, np.float32)

    # sort both edge lists by destination (row) once; un-permute outputs at the end
    seg_sp = _SegHelper(eidx[0], N)
    seg_sq = _SegHelper(seidx[0], N)
    o_sp, o_sq = seg_sp.order, seg_sq.order
    r0s, c0s = eidx[0][o_sp], eidx[1][o_sp]
    s0s, s1s = seidx[0][o_sq], seidx[1][o_sq]
    ef_s = ef[o_sp]
    nsei_s = nsei[o_sq]
    sef_s = sef[o_sq]

    ncols_e, ncols_sq, ncols_n = E // NCORES, E_SEQ // NCORES, N // NCORES

    # ---- phase 1: edge embed MLP, seq edge update MLP, latent->node MLP
    ee = p["edge_embed"]
    su = p["seq_edge_update"]
    l2n = p["lat2node"]
    edge_in = np.concatenate([ef_s, nf[r0s], nf[c0s]], axis=1)
    seq_in = np.concatenate([nsei_s, nf[s0s], nf[s1s]], axis=1)
    lat_in = np.concatenate([latent, nf], axis=1)
    ein = edge_in.shape[1]
    ch1 = [
        dict(name="ee", K0=ein, ncols=ncols_e,
             layers=[(ein, C_Z, True), (C_Z, C_Z, True), (C_Z, C_Z, False)]),
        dict(name="su", K0=ein, ncols=ncols_sq,
             layers=[(ein, C_Z, True), (C_Z, C_Z, True), (C_Z, C_Z, False)]),
        dict(name="l2n", K0=2 * C_S, ncols=ncols_n,
             layers=[(2 * C_S, C_S, True), (C_S, C_S, True), (C_S, C_S, False)]),
    ]
    w1 = {
        "ee": [_wb(ee["l1"]), _wb(ee["l2"]), _wb(ee["l3"])],
        "su": [_wb(su["l1"]), _wb(su["l2"]), _wb(su["l3"])],
        "l2n": [_wb(l2n["l1"]), _wb(l2n["l2"]), _wb(l2n["l3"])],
    }
    out1 = run_chains("p1", ch1, {"ee": _T(edge_in), "su": _T(seq_in), "l2n": _T(lat_in)}, w1)

    edge_out_s = _ln(out1["ee"].T, np.asarray(ee["ln"]["g"], np.float32),
                     np.asarray(ee["ln"]["b"], np.float32))
    seq_edge_s = _ln(sef_s + out1["su"].T, np.asarray(p["seq_edge_ln"]["g"], np.float32),
                     np.asarray(p["seq_edge_ln"]["b"], np.float32))
    node = _ln(nf + out1["l2n"].T, np.asarray(p["ln_s0"]["g"], np.float32),
               np.asarray(p["ln_s0"]["b"], np.float32))

    # ---- IPA (spatial), then IPA (sequence)
    for which, z_s, row_s, col_s, seg in (
        ("ipa_sp", edge_out_s, r0s, c0s, seg_sp),
        ("ipa_seq", seq_edge_s, s0s, s1s, seg_sq),
    ):
        ip = p[which]
        q, k, v, qp, kp, vp = _ipa_linears(which, node, z_s, ip, ncols_n)
        qp_g = _to_global(qp, rot, trans)
        kp_g = _to_global(kp, rot, trans)
        vp_g = _to_global(vp, rot, trans)
        Wb, bb_ = _wb(ip["bias"])
        b_s = z_s @ Wb + bb_
        gamma = np.log1p(np.exp(np.asarray(ip["head_w"], np.float32)))
        feats = _ipa_edge_stage(q, k, v, qp_g, kp_g, vp_g, z_s, b_s, row_s, col_s,
                                seg, rot, trans, mask, gamma)
        Wo, bo = _wb(ip["out"])
        cho = [dict(name="out", K0=feats.shape[1], ncols=ncols_n,
                    layers=[(feats.shape[1], C_S, False)])]
        upd = run_chains(("ipaout", feats.shape[1]), cho, {"out": _T(feats)},
                         {"out": [(Wo, bo)]})["out"].T
        lnp = p["ln_s1"] if which == "ipa_sp" else p["ln_s2"]
        node = _ln(node + upd * mask[:, None], np.asarray(lnp["g"], np.float32),
                   np.asarray(lnp["b"], np.float32))

    # ---- node transition
    nt = p["node_trans"]
    ch6 = [dict(name="nt", K0=C_S, ncols=ncols_n,
                layers=[(C_S, C_S, True), (C_S, C_S, True), (C_S, C_S, False)])]
    out6 = run_chains("p6", ch6, {"nt": _T(node)},
                      {"nt": [_wb(nt["l1"]), _wb(nt["l2"]), _wb(nt["l3"])]})
    node = _ln(node + out6["nt"].T, np.asarray(nt["ln"]["g"], np.float32),
               np.asarray(nt["ln"]["b"], np.float32))
    node = node * mask[:, None]

    # ---- backbone update (host: tiny) + frame compose
    Wbb, bbb = _wb(p["bb"])
    upd6 = ((node * noise[:, None]) @ Wbb + bbb) * noise[:, None]
    qv = np.concatenate([np.ones_like(upd6[:, :1]), upd6[:, :3]], axis=-1)
    qv = qv / np.linalg.norm(qv, axis=-1, keepdims=True)
    new_rot = np.einsum("nij,njk->nik", rot, _quat_to_rot(qv), optimize=True)
    new_trans = trans + np.einsum("nij,nj->ni", rot, upd6[:, 3:], optimize=True)

    # ---- edge transition (on seq edges) + node->latent
    et = p["edge_trans"]
    n2l = p["node2lat"]
    hid = C_Z + 2 * (C_S // 2)
    n2l_in = np.concatenate([latent, node], axis=1)
    ch7 = [
        dict(name="h", K0=C_S, ncols=ncols_n, layers=[(C_S, C_S // 2, False)]),
        dict(name="n2l", K0=C_S + C_LATENT, ncols=ncols_n,
             layers=[(C_S + C_LATENT, C_LATENT, True), (C_LATENT, C_LATENT, True),
                     (C_LATENT, C_LATENT, False)]),
    ]
    out7 = run_chains("p7", ch7, {"h": _T(node), "n2l": _T(n2l_in)},
                      {"h": [_wb(et["init"])],
                       "n2l": [_wb(n2l["l1"]), _wb(n2l["l2"]), _wb(n2l["l3"])]})
    h = out7["h"].T
    latent_out = latent + out7["n2l"].T

    x_et = np.concatenate([seq_edge_s, h[s0s], h[s1s]], axis=1)
    ch8 = [dict(name="et", K0=hid, ncols=ncols_sq,
                layers=[(hid, hid, True), (hid, hid, True), (hid, C_Z, False)],
                residual_before_last=True)]
    out8 = run_chains("p8", ch8, {"et": _T(x_et)},
                      {"et": [_wb(et["t1"]), _wb(et["t2"]), _wb(et["fin"])]})
    seq_final_s = _ln(out8["et"].T, np.asarray(et["ln"]["g"], np.float32),
                      np.asarray(et["ln"]["b"], np.float32))

    # un-permute edge outputs back to original edge order
    edge_out = np.empty_like(edge_out_s)
    edge_out[o_sp] = edge_out_s
    seq_final = np.empty_like(seq_final_s)
    seq_final[o_sq] = seq_final_s

    return (node.astype(np.float32), new_rot.astype(np.float32),
            new_trans.astype(np.float32), edge_out.astype(np.float32),
            seq_final.astype(np.float32), latent_out.astype(np.float32))
